# revision 27
# baseline (speedup 1.0000x reference)
"""Gemma4 MoE feed-forward on 8 Trainium2 NeuronCores.

Strategy: expert-parallel. E == n_cores == 8, so core e owns expert e's
weights (Wg[e], Wu[e], Wd[e]) and receives exactly the tokens routed to
expert e (gathered + transposed + padded on the host). Each core runs a
dense gated-FFN over its token batch:

    dT = Wd^T @ (gelu_tanh(Wg^T x^T) * (Wu^T x^T))        (all [*, C] layouts)

The host then scatter-adds routing_weight * dT^T back into the full
[T, H] output. Tokens that select the same expert in both slots are
deduplicated on the host (weights summed).

Matmul inputs and the dT output are bf16 (full PE rate, half the HBM
traffic of fp32; end-to-end rel err ~4.4e-3 on this problem); PSUM
accumulation, gelu, and the host-side combine run in fp32.

DMA discipline (each DMA instruction costs ~625 ns of HWDGE
descriptor-gen on its issuing engine, and per-partition runs need
~2-4KB for full DMA-engine rate): weights are host-pretiled so each
i-tile (and each Wd h-tile) is ONE transfer with 4KB/2KB contiguous
per partition; x is host-pretiled n-block-major so the up phase's
n=0 data loads first in ~4KB-run granules; transfers alternate
between the Sync and Scalar HWDGE queues in PE-consumption order.
"""

import os
import sys

import numpy as np

for _p in ("/opt/trn_rl_repo", "/root/.axon_site/_ro/trn_rl_repo"):
    if os.path.isdir(_p) and _p not in sys.path:
        sys.path.append(_p)

T, H, I, E, K = 4096, 2048, 1024, 8, 2
NCORES = 8
KH = H // 128  # 16 k-tiles over the hidden dim
KI = I // 128  # 8 k-tiles over the intermediate dim

_PROGRAM_CACHE = {}
LAST_RESULT = None  # BassKernelResults of the most recent run (for test.py)
TRACE = False  # test.py sets this to capture an NTFF profile
TRACE_CORES = [0]

# Dummy matmuls at launch: HAM starts throttled and promotes to 2.4 GHz
# only after sustained PE activity, and the promotion must complete before
# the first data-gated stall or the WHOLE kernel runs at ~2.0 GHz. On a
# warm device 12 suffices; from a cold/idle device the warmup runs slower
# and needs to be much longer (the ~2.4x-clock payoff dwarfs its cost).
WARMUP_MM = 30
XG = 4  # k-tiles per x DMA group


def _tile_w_up(W, bf16):
    """[H, I] -> [KI, 128, KH*128]: tile[i, p, k*128+j] = W[k*128+p, i*128+j],
    so expert i-tile i loads as ONE dma with 4KB/partition contiguous."""
    Wt = W.reshape(KH, 128, KI, 128).transpose(2, 1, 0, 3)
    return np.ascontiguousarray(Wt, dtype=bf16).reshape(KI, 128, KH * 128)


def _tile_w_down(W, bf16):
    """[I, H] -> [KH, 128, KI*128] (same scheme, contraction over I)."""
    Wt = W.reshape(KI, 128, KH, 128).transpose(2, 1, 0, 3)
    return np.ascontiguousarray(Wt, dtype=bf16).reshape(KH, 128, KI * 128)


def _pick_config(max_count):
    """Token-block config: NT blocks of even width N, NT*N >= max_count,
    N <= 512 (PSUM bank limit)."""
    mc = max(max_count, 128)
    nt = -(-mc // 512)
    n = -(-mc // nt)
    n += n % 2
    return (nt * n, nt, n)  # (C, NT, N)


def _build_program(C, NT, N):
    import concourse.tile as tile
    from concourse import bacc, mybir
    from contextlib import ExitStack

    f32 = mybir.dt.float32
    bf16 = mybir.dt.bfloat16

    nc = bacc.Bacc("TRN2", target_bir_lowering=False, debug=False)

    # x arrives host-pretiled as [NT, 128, KH*N]: n-block major so a granule
    # of G consecutive k-tiles within one n-block is G*N*2 bytes contiguous
    # per partition (~4KB at G=4) -- full DMA-engine descriptor rate.
    xTt = nc.dram_tensor("xTt", [NT, 128, KH * N], bf16, kind="ExternalInput").ap()
    Wg_d = nc.dram_tensor("Wg", [KI, 128, KH * 128], bf16, kind="ExternalInput").ap()
    Wu_d = nc.dram_tensor("Wu", [KI, 128, KH * 128], bf16, kind="ExternalInput").ap()
    Wd_d = nc.dram_tensor("Wd", [KH, 128, KI * 128], bf16, kind="ExternalInput").ap()
    dT = nc.dram_tensor("dT", [H, C], bf16, kind="ExternalOutput").ap()

    # Partition-major view: row a*128+p -> partition p, free index a.
    dT_p = dT.rearrange("(a p) c -> p a c", p=128)  # [128, KH, C]

    GELU = mybir.ActivationFunctionType.Gelu_apprx_tanh

    with tile.TileContext(nc) as tc, ExitStack() as ctx:
        xpool = ctx.enter_context(tc.tile_pool(name="x", bufs=1))
        wpool = ctx.enter_context(tc.tile_pool(name="w", bufs=3))
        wdpool = ctx.enter_context(tc.tile_pool(name="wd", bufs=1))
        apool = ctx.enter_context(tc.tile_pool(name="a", bufs=1))
        tpool = ctx.enter_context(tc.tile_pool(name="t", bufs=4))
        opool = ctx.enter_context(tc.tile_pool(name="o", bufs=4))

        # PE clock-gate warmup: HAM starts at 1.2 GHz and un-throttles only
        # after ~3.4us of sustained activity; dummy matmuls on memset scratch
        # also bridge the first weight/x DMA latency.
        with (
            tc.tile_pool(name="warm", bufs=1) as wmpool,
            tc.tile_pool(name="warmps", bufs=1, space="PSUM") as wmpspool,
        ):
            wt = wmpool.tile([128, 512], bf16, name="warm_in")
            nc.gpsimd.memset(wt[:], 0.0)
            wps = wmpspool.tile([128, 512], f32, name="warm_ps")
            for r in range(WARMUP_MM):
                nc.tensor.matmul(wps[:], wt[:, 0:128], wt[:], start=True, stop=True)

        w_tiles = {}

        def issue_w(i):
            wgt = wpool.tile([128, KH * 128], bf16, tag="wg", name=f"wg{i}")
            wut = wpool.tile([128, KH * 128], bf16, tag="wu", name=f"wu{i}")
            nc.sync.dma_start(wgt[:], Wg_d[i])
            nc.sync.dma_start(wut[:], Wu_d[i])
            w_tiles[i] = (wgt, wut)

        wd_tiles = {}

        def issue_wd(h):
            wdt = wdpool.tile([128, KI * 128], bf16, tag=f"wd{h}", name=f"wd{h}")
            nc.sync.dma_start(wdt[:], Wd_d[h])
            wd_tiles[h] = wdt

        # Head schedule. The first ~25us are DMA-bandwidth-floor-bound (i=0
        # weights + all of x + i=1 weights = 6MB), so granules are emitted in
        # exact PE-consumption order -- n=0 block first -- alternating between
        # the two HWDGE queues (each gets ~half the DMA engines while both
        # are busy). All granules keep >=4KB-class per-partition runs.
        NXG = KH // XG
        wgt0 = wpool.tile([128, KH * 128], bf16, tag="wg", name="wg0")
        wut0 = wpool.tile([128, KH * 128], bf16, tag="wu", name="wu0")
        wgt1 = wpool.tile([128, KH * 128], bf16, tag="wg", name="wg1")
        wut1 = wpool.tile([128, KH * 128], bf16, tag="wu", name="wu1")
        xns = [xpool.tile([128, KH * N], bf16, name=f"xn{n}") for n in range(NT)]

        def x_gran(k0, k1, n):
            rs = slice(k0 * N, k1 * N)
            return lambda eng: eng.dma_start(xns[n][:, rs], xTt[n, :, rs])

        # The very first g-matmul needs only wg0 + x k0-1; wu0 is needed one
        # matmul later. Emit in strict first-use order so the PE can leave
        # warmup with zero stall at ~240GB/s early DMA delivery.
        head = [
            lambda eng: eng.dma_start(wgt0[:], Wg_d[0]),
            x_gran(0, 2, 0),
            x_gran(2, 4, 0),
            lambda eng: eng.dma_start(wut0[:], Wu_d[0]),
            x_gran(4, 6, 0),
            x_gran(6, 8, 0),
        ]
        head += [x_gran(j * XG, (j + 1) * XG, 0) for j in range(2, NXG)]
        for n in range(1, NT):
            head += [x_gran(j * XG, (j + 1) * XG, n) for j in range(NXG)]
        head += [
            lambda eng: eng.dma_start(wgt1[:], Wg_d[1]),
            lambda eng: eng.dma_start(wut1[:], Wu_d[1]),
        ]
        for gi, fn in enumerate(head):
            fn(nc.sync if gi % 2 == 0 else nc.scalar)
        w_tiles[0] = (wgt0, wut0)
        w_tiles[1] = (wgt1, wut1)

        def xk(k, n):
            return xns[n][:, k * N : (k + 1) * N]

        aT = apool.tile([128, KI, C], bf16, name="aT")

        # All 16 down-weight fetches are spread over the up phase, so the
        # down phase starts with every Wd tile resident.
        wd_sched = {2: [0, 1, 2], 3: [3, 4, 5], 4: [6, 7, 8], 5: [9, 10, 11],
                    6: [12, 13, 14], 7: [15]}

        d_bufs = 2 if 4 + 2 * NT <= 8 else 1
        with (
            tc.tile_pool(name="gu", bufs=2, space="PSUM") as gupool,
            tc.tile_pool(name="d", bufs=d_bufs, space="PSUM") as dpool,
        ):
            for i in range(KI):
                if i + 2 < KI:
                    issue_w(i + 2)
                for h in wd_sched.get(i, []):
                    issue_wd(h)
                wgt, wut = w_tiles.pop(i)
                for n in range(NT):
                    nsl = slice(n * N, (n + 1) * N)
                    g_ps = gupool.tile([128, N], f32, tag="g", name=f"g{i}_{n}")
                    u_ps = gupool.tile([128, N], f32, tag="u", name=f"u{i}_{n}")
                    for k in range(KH):
                        ksl = slice(k * 128, (k + 1) * 128)
                        nc.tensor.matmul(
                            g_ps[:], wgt[:, ksl], xk(k, n),
                            start=(k == 0), stop=(k == KH - 1),
                        )
                        nc.tensor.matmul(
                            u_ps[:], wut[:, ksl], xk(k, n),
                            start=(k == 0), stop=(k == KH - 1),
                        )
                    gel = tpool.tile([128, N], f32, tag="gelu", name=f"gel{i}_{n}")
                    nc.scalar.activation(gel[:], g_ps[:], GELU)
                    nc.vector.tensor_mul(aT[:, i, nsl], gel[:], u_ps[:])

            for h in range(KH):
                if h not in wd_tiles:
                    issue_wd(h)
                wdt = wd_tiles.pop(h)
                if h < KH - 1:
                    d_ps = [
                        dpool.tile([128, N], f32, tag=f"d{n}", name=f"d{h}_{n}")
                        for n in range(NT)
                    ]
                    for ki in range(KI):
                        lw = wdt[:, ki * 128 : (ki + 1) * 128]
                        for n in range(NT):
                            nc.tensor.matmul(
                                d_ps[n][:], lw, aT[:, ki, n * N : (n + 1) * N],
                                start=(ki == 0), stop=(ki == KI - 1),
                            )
                    for n in range(NT):
                        o = opool.tile([128, N], bf16, tag="o", name=f"o{h}_{n}")
                        nc.vector.tensor_copy(o[:], d_ps[n][:])
                        eng = nc.sync if n % 2 == 0 else nc.scalar
                        eng.dma_start(dT_p[:, h, n * N : (n + 1) * N], o[:])
                else:
                    # Last h: finish the n-chains one at a time, each split
                    # into column-half accumulation chains, so the first
                    # half's copy+DMA overlaps the second half's matmuls and
                    # the post-final-matmul tail is one short transfer split
                    # over both HWDGE queues.
                    M = N // 2
                    for n in range(NT):
                        o = opool.tile([128, N], bf16, tag="o", name=f"o{h}_{n}")
                        for c in range(2):
                            cw = (N - M) if c else M
                            base = n * N + c * M
                            # separate PSUM tags -> different banks, so the
                            # first half's copy overlaps the second's matmuls
                            dh = dpool.tile(
                                [128, cw], f32, tag=f"d{c}", name=f"d{h}_{n}_{c}"
                            )
                            for ki in range(KI):
                                nc.tensor.matmul(
                                    dh[:],
                                    wdt[:, ki * 128 : (ki + 1) * 128],
                                    aT[:, ki, base : base + cw],
                                    start=(ki == 0), stop=(ki == KI - 1),
                                )
                            csl = slice(c * M, c * M + cw)
                            nc.vector.tensor_copy(o[:, csl], dh[:])
                            nc.sync.dma_start(
                                dT_p[0:64, h, base : base + cw], o[0:64, csl]
                            )
                            nc.scalar.dma_start(
                                dT_p[64:128, h, base : base + cw], o[64:128, csl]
                            )

    nc.compile()
    return nc


def _get_program(C, NT, N):
    key = (C, NT, N)
    if key not in _PROGRAM_CACHE:
        _PROGRAM_CACHE[key] = _build_program(C, NT, N)
    return _PROGRAM_CACHE[key]


def _ensure_ntff_hook():
    """Register the axon NTFF profile hook if the image's antenv lacks
    axon_hooks (see trn_agent_boot.trn_boot). Only needed when TRACE."""
    import types

    try:
        from antenv.axon_hooks import get_axon_ntff_profile_hook  # noqa: F401

        return
    except ImportError:
        pass
    import antenv
    from trn_agent_boot.trn_boot import _ntff_profile_via_ctypes

    hook = _ntff_profile_via_ctypes("/opt/axon/libaxon_pjrt.so")
    mod = types.ModuleType("antenv.axon_hooks")
    state = {"hook": hook}
    mod.set_axon_ntff_profile_hook = lambda h: state.__setitem__("hook", h)
    mod.get_axon_ntff_profile_hook = lambda: state["hook"]
    sys.modules["antenv.axon_hooks"] = mod
    antenv.axon_hooks = mod


def kernel(x, Wg, Wu, Wd, selected_experts, routing_weights):
    global LAST_RESULT
    import ml_dtypes
    from concourse.bass_utils import run_bass_kernel_spmd

    if TRACE:
        _ensure_ntff_hook()

    bf16 = ml_dtypes.bfloat16

    x = np.asarray(x, dtype=np.float32)
    Wg = np.asarray(Wg, dtype=np.float32)
    Wu = np.asarray(Wu, dtype=np.float32)
    Wd = np.asarray(Wd, dtype=np.float32)
    selected_experts = np.asarray(selected_experts)
    routing_weights = np.asarray(routing_weights, dtype=np.float32)

    # Host-side dispatch: per expert, the (deduplicated) token list and
    # summed routing weights.
    idx_list, w_list = [], []
    for e in range(E):
        m = selected_experts == e  # [T, K]
        idx = np.nonzero(m.any(axis=1))[0]
        w = (routing_weights * m).sum(axis=1)[idx]
        idx_list.append(idx)
        w_list.append(w.astype(np.float32))

    max_count = max(len(idx) for idx in idx_list)
    C, NT, N = _pick_config(max_count)

    nc = _get_program(C, NT, N)

    in_maps = []
    for e in range(E):
        idx = idx_list[e]
        xT = np.zeros((H, C), dtype=bf16)
        xT[:, : len(idx)] = np.ascontiguousarray(x[idx].T, dtype=bf16)
        # [H, C] -> [NT, 128, KH*N]: n-block major, 4KB-class contiguous runs
        xTt = np.ascontiguousarray(
            xT.reshape(KH, 128, NT, N).transpose(2, 1, 0, 3)
        ).reshape(NT, 128, KH * N)
        in_maps.append(
            {
                "xTt": xTt,
                "Wg": _tile_w_up(Wg[e], bf16),
                "Wu": _tile_w_up(Wu[e], bf16),
                "Wd": _tile_w_down(Wd[e], bf16),
            }
        )

    # The device sporadically dies mid-execution (NRT_EXEC_UNIT_UNRECOVERABLE,
    # ~10% of runs observed); one retry with a core reset usually recovers.
    try:
        res = run_bass_kernel_spmd(
            nc,
            in_maps,
            list(range(NCORES)),
            trace=TRACE,
            trace_cores=TRACE_CORES if TRACE else None,
        )
    except Exception:
        os.environ["NEURON_RT_RESET_CORES"] = "1"
        res = run_bass_kernel_spmd(
            nc,
            in_maps,
            list(range(NCORES)),
            trace=TRACE,
            trace_cores=TRACE_CORES if TRACE else None,
        )
    LAST_RESULT = res

    out = np.zeros((T, H), dtype=np.float32)
    for e in range(E):
        idx = idx_list[e]
        dTe = np.asarray(res.results[e]["dT"], dtype=np.float32)  # [H, C]
        out[idx] += w_list[e][:, None] * dTe[:, : len(idx)].T
    return out
